# revision 13
# baseline (speedup 1.0000x reference)
"""Multi-head attention (B=2, S=2048, H=32, D=128) on 8 Trainium2 NeuronCores.

Sharding: tensor-parallel across heads.
  - core c owns heads [4c, 4c+4) (= feature slice [512c, 512(c+1)) of the
    projected dim) for BOTH batches.
  - Each core projects q/k/v (weights column-sharded by head) for all
    4096 tokens, runs attention for its 4 heads (batches never mix), then
    an 8-core AllToAll (split 2/1/1 over heads so transfers overlap the
    attention tail and the output projection start) reshards the context
    from head-major to token-major (512 tokens per core, global token
    index = b*2048 + s). Each core runs the full output projection on its
    token block, producing o^T [4096, 512]; host transposes + concatenates.

Layouts (per core, all device compute in bf16 with fp32 PSUM accumulation):
  - inputs arrive pre-transposed feature-major: qT/kT/vT [4096, 4096]
  - qpT/kpT [128, 4 heads, 4096 tokens] feature-major
  - vp [128, 32 token-tiles, 512 features] token-major
  - scores computed k-major S^T[k, q] in 2-bank PSUM tiles so each Exp
    activation covers 1024 columns (amortizes ACT instruction overhead);
    exp(P^T) feeds the ctx matmul directly as rhs (no transposes anywhere)
  - softmax row sums: DVE adds the 16 P^T tiles, then one ones[128,128]
    matmul fuses the partition-sum with the broadcast of s across
    partitions; 1/s (reciprocal_approx_fast) applied during ctx evacuation.
"""

import numpy as np
import ml_dtypes

import concourse.bacc as bacc
import concourse.mybir as mybir
import concourse.tile as tile
from concourse.bass_utils import run_bass_kernel_spmd

bf16 = ml_dtypes.bfloat16

B, S, H, D = 2, 2048, 32, 128
DM = H * D                      # 4096
BT = B * S                      # 4096 tokens total
N_CORES = 8
HL = H // N_CORES               # heads per core = 4
FL = HL * D                     # feature slice per core = 512
TB = BT // N_CORES              # output token block per core = 512
SCALE = float(D) ** -0.5

F32 = mybir.dt.float32
BF16 = mybir.dt.bfloat16

_CACHE = {}


def _build():
    nc = bacc.Bacc("TRN2", target_bir_lowering=False, debug=False,
                   num_devices=N_CORES)

    qT = nc.dram_tensor("qT", [DM, BT], BF16, kind="ExternalInput")
    kT = nc.dram_tensor("kT", [DM, BT], BF16, kind="ExternalInput")
    vT = nc.dram_tensor("vT", [DM, BT], BF16, kind="ExternalInput")
    wqT = nc.dram_tensor("wqT", [DM, FL], BF16, kind="ExternalInput")
    wkT = nc.dram_tensor("wkT", [DM, FL], BF16, kind="ExternalInput")
    wvT = nc.dram_tensor("wvT", [DM, FL], BF16, kind="ExternalInput")
    woT = nc.dram_tensor("woT", [DM, DM], BF16, kind="ExternalInput")
    outT = nc.dram_tensor("outT", [DM, TB], F32, kind="ExternalOutput")

    NKT = S // 128              # 16 k token-tiles per sequence
    Exp = mybir.ActivationFunctionType.Exp
    # AllToAll split over local heads: {0,1}, {2}, {3}
    A2A_H = [(0, 2), (2, 3), (3, 4)]

    with tile.TileContext(nc) as tc:
        with (
            tc.tile_pool(name="big", bufs=2, space="PSUM") as bigp,
            tc.tile_pool(name="ps", bufs=4, space="PSUM") as psp,
            tc.tile_pool(name="dram", bufs=1, space="DRAM") as dram,
            tc.tile_pool(name="ctxgp", bufs=1) as ctxgp,
        ):
            persist_cm = tc.tile_pool(name="persist", bufs=1)
            persist = persist_cm.__enter__()
            qpT = persist.tile([128, HL, BT], BF16, tag="qpT")
            kpT = persist.tile([128, HL, BT], BF16, tag="kpT")
            vp = persist.tile([128, BT // 128, FL], BF16, tag="vp")

            ones_m = persist.tile([128, 128], BF16, tag="ones_m")
            nc.vector.memset(ones_m[:], 1.0)

            # ---------------- phase 1: projections ----------------
            with tc.tile_pool(name="proj", bufs=2) as proj:
                for x_dram, w_dram, feat_major, out_t in (
                    (qT, wqT, True, qpT),
                    (kT, wkT, True, kpT),
                    (vT, wvT, False, vp),
                ):
                    x_r = x_dram.ap().rearrange("(kk p) t -> p kk t", p=128)
                    w_r = w_dram.ap().rearrange("(kk p) f -> p kk f", p=128)
                    wh = proj.tile([128, 32, FL], BF16, tag="w", bufs=1)
                    # split the weight load so the first matmuls start sooner
                    for wq_ in range(8):
                        nc.sync.dma_start(
                            out=wh[:, 4 * wq_:4 * (wq_ + 1), :],
                            in_=w_r[:, 4 * wq_:4 * (wq_ + 1), :])
                    for tch in range(8):       # 512-token chunks
                        pss = [bigp.tile([128, 1024], F32, tag="big",
                                         name=f"pss{mp}")
                               for mp in range(2)]
                        for kh in range(2):    # halves of the contraction
                            xs = proj.tile([128, 16, 512], BF16, tag="xs")
                            nc.sync.dma_start(
                                out=xs[:],
                                in_=x_r[:, kh * 16:(kh + 1) * 16,
                                        tch * 512:(tch + 1) * 512])
                            for ms in range(4):
                                dst = pss[ms // 2][:, (ms % 2) * 512:
                                                   (ms % 2 + 1) * 512]
                                for kk in range(16):
                                    first = (kh == 0 and kk == 0)
                                    last = (kh == 1 and kk == 15)
                                    if feat_major:
                                        # out[f_out, t] += W^T.T @ xT
                                        nc.tensor.matmul(
                                            dst,
                                            wh[:, kh * 16 + kk,
                                               ms * 128:(ms + 1) * 128],
                                            xs[:, kk, :],
                                            start=first, stop=last)
                                    else:
                                        # out[t, f_out] += xT.T @ W^T
                                        nc.tensor.matmul(
                                            dst,
                                            xs[:, kk,
                                               ms * 128:(ms + 1) * 128],
                                            wh[:, kh * 16 + kk, :],
                                            start=first, stop=last)
                        for mp in range(2):
                            if feat_major:
                                dst = out_t[:, 2 * mp:2 * mp + 2,
                                            tch * 512:(tch + 1) * 512]
                            else:
                                dst = out_t[:, tch * 4 + 2 * mp:
                                            tch * 4 + 2 * mp + 2, :]
                            nc.vector.tensor_copy(dst, pss[mp][:])

            # ---------------- phase 2: attention ----------------
            in_bufs, out_bufs = [], []
            for gi, (h0, h1) in enumerate(A2A_H):
                in_bufs.append(dram.tile([N_CORES, h1 - h0, 128, TB], BF16,
                                         name=f"a2a_in{gi}"))
                out_bufs.append(dram.tile([N_CORES, h1 - h0, 128, TB], BF16,
                                          name=f"a2a_out{gi}"))

            ctxg = []
            with tc.tile_pool(name="attn", bufs=2) as attn:
                for hl in range(HL):
                    for b in range(B):
                        for qb in range(4):    # 512-query blocks
                            qs = slice(b * S + qb * TB, b * S + (qb + 1) * TB)
                            pt = attn.tile([128, NKT, TB], BF16, tag="pt",
                                           bufs=3)
                            for g in range(NKT // 2):   # k-tile pairs
                                st = bigp.tile([128, 1024], F32, tag="big")
                                for half in range(2):
                                    kt = 2 * g + half
                                    # S^T[k_tok, q] = khT.T @ qhT
                                    nc.tensor.matmul(
                                        st[:, half * 512:(half + 1) * 512],
                                        kpT[:, hl, b * S + kt * 128:
                                            b * S + (kt + 1) * 128],
                                        qpT[:, hl, qs],
                                        start=True, stop=True)
                                nc.scalar.activation(
                                    pt[:, 2 * g:2 * g + 2, :], st[:],
                                    Exp, scale=SCALE)
                            # partial row sums over the 16 k-tiles (DVE)
                            sp2 = attn.tile([128, 2, TB], BF16, tag="sp2")
                            nc.vector.tensor_add(sp2[:], pt[:, 0:2, :],
                                                 pt[:, 2:4, :])
                            for g in range(2, NKT // 2):
                                nc.vector.tensor_add(sp2[:], sp2[:],
                                                     pt[:, 2 * g:2 * g + 2, :])
                            sp = attn.tile([128, TB], BF16, tag="sp")
                            nc.vector.tensor_add(sp[:], sp2[:, 0, :],
                                                 sp2[:, 1, :])
                            # fused partition-sum + broadcast: ones.T @ sp
                            ps_b = psp.tile([128, TB], F32, tag="mm")
                            nc.tensor.matmul(ps_b[:], ones_m[:], sp[:],
                                             start=True, stop=True)
                            rsb = attn.tile([128, TB], F32, tag="rsb")
                            nc.vector.reciprocal_approx_fast(rsb[:], ps_b[:])
                            # ctx^T[d, q] = sum_kt vh[kt].T @ P^T[kt]
                            ps_c = psp.tile([128, TB], F32, tag="mm")
                            for kt in range(NKT):
                                nc.tensor.matmul(
                                    ps_c[:],
                                    vp[:, b * NKT + kt,
                                       hl * 128:(hl + 1) * 128],
                                    pt[:, kt, :],
                                    start=(kt == 0), stop=(kt == NKT - 1))
                            ctxs = attn.tile([128, TB], BF16, tag="ctxs")
                            nc.vector.tensor_tensor(
                                ctxs[:], ps_c[:], rsb[:],
                                op=mybir.AluOpType.mult)
                            for gi, (h0, h1) in enumerate(A2A_H):
                                if h0 <= hl < h1:
                                    nc.sync.dma_start(
                                        out=in_bufs[gi][b * 4 + qb, hl - h0],
                                        in_=ctxs[:])
                    for gi, (h0, h1) in enumerate(A2A_H):
                        if hl == h1 - 1:
                            nc.gpsimd.collective_compute(
                                "AllToAll", mybir.AluOpType.bypass,
                                replica_groups=[list(range(N_CORES))],
                                ins=[in_bufs[gi].opt()],
                                outs=[out_bufs[gi].opt()])
                            cg = ctxgp.tile(
                                [128, N_CORES * (h1 - h0), TB], BF16,
                                tag=f"ctxg{gi}", name=f"ctxg{gi}")
                            nc.sync.dma_start(
                                out=cg[:],
                                in_=out_bufs[gi].rearrange(
                                    "j h p t -> p (j h) t"))
                            ctxg.append(cg)

            persist_cm.__exit__(None, None, None)

            # ---------------- phase 3: output projection ----------------
            # o^T[f_out, t] = sum over the 32 ctx feature tiles
            # global feature tile kk = j*HL + hl  (j = source rank)
            with tc.tile_pool(name="oproj", bufs=2) as op:
                # accumulation order: all group-0 tiles, then 1, then 2 so
                # early matmuls run while later AllToAlls are in flight
                mm_seq = []
                for gi, (h0, h1) in enumerate(A2A_H):
                    for j in range(N_CORES):
                        for hl in range(h0, h1):
                            kk = j * HL + hl
                            mm_seq.append(
                                (kk, ctxg[gi], j * (h1 - h0) + hl - h0))

                wo_r = woT.ap().rearrange("(kk p) f -> p kk f", p=128)
                for fo in range(32):           # 128-wide out-feature tiles
                    woc = op.tile([128, 32, 128], BF16, tag="woc")
                    nc.sync.dma_start(
                        out=woc[:], in_=wo_r[:, :, fo * 128:(fo + 1) * 128])
                    ps_o = psp.tile([128, TB], F32, tag="mm")
                    for n_mm, (kk, cg, ci) in enumerate(mm_seq):
                        nc.tensor.matmul(
                            ps_o[:],
                            woc[:, kk, :],
                            cg[:, ci, :],
                            start=(n_mm == 0), stop=(n_mm == 31))
                    ot = op.tile([128, TB], F32, tag="ot")
                    nc.vector.tensor_copy(ot[:], ps_o[:])
                    nc.sync.dma_start(
                        out=outT[fo * 128:(fo + 1) * 128, :], in_=ot[:])

    nc.compile()
    return nc


def _prep_inputs(q, k, v, Wq, Wk, Wv, Wo):
    """Host-side sharding: cast to bf16, transpose to feature-major, slice."""
    q, k, v = (np.asarray(x, dtype=np.float32) for x in (q, k, v))
    Wq, Wk, Wv, Wo = (np.asarray(x, dtype=np.float32)
                      for x in (Wq, Wk, Wv, Wo))
    qT = np.ascontiguousarray(q.reshape(BT, DM).astype(bf16).T)
    kT = np.ascontiguousarray(k.reshape(BT, DM).astype(bf16).T)
    vT = np.ascontiguousarray(v.reshape(BT, DM).astype(bf16).T)
    woT = np.ascontiguousarray(Wo.astype(bf16).T)
    in_maps = []
    for c in range(N_CORES):
        sl = slice(c * FL, (c + 1) * FL)
        in_maps.append({
            "qT": qT, "kT": kT, "vT": vT,
            "wqT": np.ascontiguousarray(Wq[sl, :].astype(bf16).T),
            "wkT": np.ascontiguousarray(Wk[sl, :].astype(bf16).T),
            "wvT": np.ascontiguousarray(Wv[sl, :].astype(bf16).T),
            "woT": woT,
        })
    return in_maps


def run_spmd(inputs, trace=False):
    if "nc" not in _CACHE:
        _CACHE["nc"] = _build()
    nc = _CACHE["nc"]
    in_maps = _prep_inputs(**inputs)
    res = run_bass_kernel_spmd(nc, in_maps, core_ids=list(range(N_CORES)),
                               trace=trace)
    o = np.empty((BT, DM), dtype=np.float32)
    for c in range(N_CORES):
        o[c * TB:(c + 1) * TB, :] = res.results[c]["outT"].T
    return o.reshape(B, S, DM), res


def kernel(q, k, v, Wq, Wk, Wv, Wo):
    o, _ = run_spmd(dict(q=q, k=k, v=v, Wq=Wq, Wk=Wk, Wv=Wv, Wo=Wo))
    return o


# revision 14
# speedup vs baseline: 1.0383x; 1.0383x over previous
"""Multi-head attention (B=2, S=2048, H=32, D=128) on 8 Trainium2 NeuronCores.

Sharding: tensor-parallel across heads.
  - core c owns heads [4c, 4c+4) (= feature slice [512c, 512(c+1)) of the
    projected dim) for BOTH batches.
  - Each core projects q/k/v (weights column-sharded by head) for all
    4096 tokens, runs attention for its 4 heads (batches never mix), then
    an 8-core AllToAll (split 2/1/1 over heads so transfers overlap the
    attention tail and the output projection start) reshards the context
    from head-major to token-major (512 tokens per core, global token
    index = b*2048 + s). Each core runs the full output projection on its
    token block, producing o^T [4096, 512]; host transposes + concatenates.

Layouts (per core, all device compute in bf16 with fp32 PSUM accumulation):
  - inputs arrive pre-transposed feature-major: qT/kT/vT [4096, 4096]
  - qpT/kpT [128, 4 heads, 4096 tokens] feature-major
  - vp [128, 32 token-tiles, 512 features] token-major
  - scores computed k-major S^T[k, q] in 2-bank PSUM tiles so each Exp
    activation covers 1024 columns (amortizes ACT instruction overhead);
    exp(P^T) feeds the ctx matmul directly as rhs (no transposes anywhere)
  - softmax row sums: DVE adds the 16 P^T tiles, then one ones[128,128]
    matmul fuses the partition-sum with the broadcast of s across
    partitions; 1/s (reciprocal_approx_fast) applied during ctx evacuation.
"""

import numpy as np
import ml_dtypes

import concourse.bacc as bacc
import concourse.mybir as mybir
import concourse.tile as tile
from concourse.bass_utils import run_bass_kernel_spmd

bf16 = ml_dtypes.bfloat16

B, S, H, D = 2, 2048, 32, 128
DM = H * D                      # 4096
BT = B * S                      # 4096 tokens total
N_CORES = 8
HL = H // N_CORES               # heads per core = 4
FL = HL * D                     # feature slice per core = 512
TB = BT // N_CORES              # output token block per core = 512
SCALE = float(D) ** -0.5

F32 = mybir.dt.float32
BF16 = mybir.dt.bfloat16

_CACHE = {}


def _build():
    nc = bacc.Bacc("TRN2", target_bir_lowering=False, debug=False,
                   num_devices=N_CORES)

    qT = nc.dram_tensor("qT", [DM, BT], BF16, kind="ExternalInput")
    kT = nc.dram_tensor("kT", [DM, BT], BF16, kind="ExternalInput")
    vT = nc.dram_tensor("vT", [DM, BT], BF16, kind="ExternalInput")
    wqT = nc.dram_tensor("wqT", [DM, FL], BF16, kind="ExternalInput")
    wkT = nc.dram_tensor("wkT", [DM, FL], BF16, kind="ExternalInput")
    wvT = nc.dram_tensor("wvT", [DM, FL], BF16, kind="ExternalInput")
    woT = nc.dram_tensor("woT", [DM, DM], BF16, kind="ExternalInput")
    outT = nc.dram_tensor("outT", [DM, TB], F32, kind="ExternalOutput")

    NKT = S // 128              # 16 k token-tiles per sequence
    Exp = mybir.ActivationFunctionType.Exp
    # AllToAll split over local heads: {0,1}, {2}, {3}
    A2A_H = [(0, 2), (2, 3), (3, 4)]

    with tile.TileContext(nc) as tc:
        with (
            tc.tile_pool(name="persist", bufs=1) as persist,
            tc.tile_pool(name="big", bufs=3, space="PSUM") as bigp,
            tc.tile_pool(name="ps", bufs=2, space="PSUM") as psp,
            tc.tile_pool(name="dram", bufs=1, space="DRAM") as dram,
        ):
            qpT = persist.tile([128, HL, BT], BF16, tag="qpT")
            kpT = persist.tile([128, HL, BT], BF16, tag="kpT")
            vp = persist.tile([128, BT // 128, FL], BF16, tag="vp")

            ones_m = persist.tile([128, 128], BF16, tag="ones_m")
            nc.vector.memset(ones_m[:], 1.0)

            # ---------------- phase 1: projections ----------------
            with tc.tile_pool(name="proj", bufs=2) as proj:
                for x_dram, w_dram, feat_major, out_t in (
                    (qT, wqT, True, qpT),
                    (kT, wkT, True, kpT),
                    (vT, wvT, False, vp),
                ):
                    x_r = x_dram.ap().rearrange("(kk p) t -> p kk t", p=128)
                    w_r = w_dram.ap().rearrange("(kk p) f -> p kk f", p=128)
                    wh = proj.tile([128, 32, FL], BF16, tag="w")
                    # split the weight load so the first matmuls start sooner
                    nc.sync.dma_start(out=wh[:, 0:16, :],
                                      in_=w_r[:, 0:16, :])
                    nc.sync.dma_start(out=wh[:, 16:32, :],
                                      in_=w_r[:, 16:32, :])
                    for tch in range(8):       # 512-token chunks
                        pss = [bigp.tile([128, 1024], F32, tag="big",
                                         name=f"pss{mp}")
                               for mp in range(2)]
                        for kh in range(2):    # halves of the contraction
                            xs = proj.tile([128, 16, 512], BF16, tag="xs")
                            nc.sync.dma_start(
                                out=xs[:],
                                in_=x_r[:, kh * 16:(kh + 1) * 16,
                                        tch * 512:(tch + 1) * 512])
                            for ms in range(4):
                                dst = pss[ms // 2][:, (ms % 2) * 512:
                                                   (ms % 2 + 1) * 512]
                                for kk in range(16):
                                    first = (kh == 0 and kk == 0)
                                    last = (kh == 1 and kk == 15)
                                    if feat_major:
                                        # out[f_out, t] += W^T.T @ xT
                                        nc.tensor.matmul(
                                            dst,
                                            wh[:, kh * 16 + kk,
                                               ms * 128:(ms + 1) * 128],
                                            xs[:, kk, :],
                                            start=first, stop=last)
                                    else:
                                        # out[t, f_out] += xT.T @ W^T
                                        nc.tensor.matmul(
                                            dst,
                                            xs[:, kk,
                                               ms * 128:(ms + 1) * 128],
                                            wh[:, kh * 16 + kk, :],
                                            start=first, stop=last)
                        for mp in range(2):
                            if feat_major:
                                dst = out_t[:, 2 * mp:2 * mp + 2,
                                            tch * 512:(tch + 1) * 512]
                            else:
                                dst = out_t[:, tch * 4 + 2 * mp:
                                            tch * 4 + 2 * mp + 2, :]
                            nc.vector.tensor_copy(dst, pss[mp][:])

            # ---------------- phase 2: attention ----------------
            in_bufs, out_bufs = [], []
            for gi, (h0, h1) in enumerate(A2A_H):
                in_bufs.append(dram.tile([N_CORES, h1 - h0, 128, TB], BF16,
                                         name=f"a2a_in{gi}"))
                out_bufs.append(dram.tile([N_CORES, h1 - h0, 128, TB], BF16,
                                          name=f"a2a_out{gi}"))

            with tc.tile_pool(name="attn", bufs=2) as attn:
                for hl in range(HL):
                    for b in range(B):
                        for qb in range(4):    # 512-query blocks
                            qs = slice(b * S + qb * TB, b * S + (qb + 1) * TB)
                            pt = attn.tile([128, NKT, TB], BF16, tag="pt")
                            for g in range(NKT // 2):   # k-tile pairs
                                st = bigp.tile([128, 1024], F32, tag="big")
                                for half in range(2):
                                    kt = 2 * g + half
                                    # S^T[k_tok, q] = khT.T @ qhT
                                    nc.tensor.matmul(
                                        st[:, half * 512:(half + 1) * 512],
                                        kpT[:, hl, b * S + kt * 128:
                                            b * S + (kt + 1) * 128],
                                        qpT[:, hl, qs],
                                        start=True, stop=True)
                                nc.scalar.activation(
                                    pt[:, 2 * g:2 * g + 2, :], st[:],
                                    Exp, scale=SCALE)
                            # partial row sums over the 16 k-tiles (DVE)
                            sp2 = attn.tile([128, 2, TB], BF16, tag="sp2")
                            nc.vector.tensor_add(sp2[:], pt[:, 0:2, :],
                                                 pt[:, 2:4, :])
                            for g in range(2, NKT // 2):
                                nc.vector.tensor_add(sp2[:], sp2[:],
                                                     pt[:, 2 * g:2 * g + 2, :])
                            sp = attn.tile([128, TB], BF16, tag="sp")
                            nc.vector.tensor_add(sp[:], sp2[:, 0, :],
                                                 sp2[:, 1, :])
                            # fused partition-sum + broadcast: ones.T @ sp
                            ps_b = psp.tile([128, TB], F32, tag="mm")
                            nc.tensor.matmul(ps_b[:], ones_m[:], sp[:],
                                             start=True, stop=True)
                            rsb = attn.tile([128, TB], F32, tag="rsb")
                            nc.vector.reciprocal_approx_fast(rsb[:], ps_b[:])
                            # ctx^T[d, q] = sum_kt vh[kt].T @ P^T[kt]
                            ps_c = psp.tile([128, TB], F32, tag="mm")
                            for kt in range(NKT):
                                nc.tensor.matmul(
                                    ps_c[:],
                                    vp[:, b * NKT + kt,
                                       hl * 128:(hl + 1) * 128],
                                    pt[:, kt, :],
                                    start=(kt == 0), stop=(kt == NKT - 1))
                            ctxs = attn.tile([128, TB], BF16, tag="ctxs")
                            nc.vector.tensor_tensor(
                                ctxs[:], ps_c[:], rsb[:],
                                op=mybir.AluOpType.mult)
                            for gi, (h0, h1) in enumerate(A2A_H):
                                if h0 <= hl < h1:
                                    nc.sync.dma_start(
                                        out=in_bufs[gi][b * 4 + qb, hl - h0],
                                        in_=ctxs[:])
                    for gi, (h0, h1) in enumerate(A2A_H):
                        if hl == h1 - 1:
                            nc.gpsimd.collective_compute(
                                "AllToAll", mybir.AluOpType.bypass,
                                replica_groups=[list(range(N_CORES))],
                                ins=[in_bufs[gi].opt()],
                                outs=[out_bufs[gi].opt()])

            # ---------------- phase 3: output projection ----------------
            # o^T[f_out, t] = sum over the 32 ctx feature tiles
            # global feature tile kk = j*HL + hl  (j = source rank)
            with tc.tile_pool(name="oproj", bufs=2) as op:
                ctxg = []
                for gi, (h0, h1) in enumerate(A2A_H):
                    cg = op.tile([128, N_CORES * (h1 - h0), TB], BF16,
                                 tag=f"ctxg{gi}", name=f"ctxg{gi}")
                    nc.sync.dma_start(
                        out=cg[:],
                        in_=out_bufs[gi].rearrange("j h p t -> p (j h) t"))
                    ctxg.append(cg)
                # accumulation order: all group-0 tiles, then 1, then 2 so
                # early matmuls run while later AllToAlls are in flight
                mm_seq = []
                for gi, (h0, h1) in enumerate(A2A_H):
                    for j in range(N_CORES):
                        for hl in range(h0, h1):
                            kk = j * HL + hl
                            mm_seq.append(
                                (kk, ctxg[gi], j * (h1 - h0) + hl - h0))

                wo_r = woT.ap().rearrange("(kk p) f -> p kk f", p=128)
                for fop in range(16):          # 256-wide out-feature pairs
                    woc = op.tile([128, 32, 256], BF16, tag="woc")
                    nc.sync.dma_start(
                        out=woc[:], in_=wo_r[:, :, fop * 256:(fop + 1) * 256])
                    ps_o = bigp.tile([128, 1024], F32, tag="big")
                    for sub in range(2):
                        dst = ps_o[:, sub * 512:(sub + 1) * 512]
                        for n_mm, (kk, cg, ci) in enumerate(mm_seq):
                            nc.tensor.matmul(
                                dst,
                                woc[:, kk, sub * 128:(sub + 1) * 128],
                                cg[:, ci, :],
                                start=(n_mm == 0), stop=(n_mm == 31))
                    ot = op.tile([128, 2, TB], F32, tag="ot")
                    nc.vector.tensor_copy(ot[:], ps_o[:])
                    nc.sync.dma_start(
                        out=outT.ap().rearrange(
                            "(fo p) t -> p fo t", p=128)[
                            :, fop * 2:fop * 2 + 2, :],
                        in_=ot[:])

    nc.compile()
    return nc


def _prep_inputs(q, k, v, Wq, Wk, Wv, Wo):
    """Host-side sharding: cast to bf16, transpose to feature-major, slice."""
    q, k, v = (np.asarray(x, dtype=np.float32) for x in (q, k, v))
    Wq, Wk, Wv, Wo = (np.asarray(x, dtype=np.float32)
                      for x in (Wq, Wk, Wv, Wo))
    qT = np.ascontiguousarray(q.reshape(BT, DM).astype(bf16).T)
    kT = np.ascontiguousarray(k.reshape(BT, DM).astype(bf16).T)
    vT = np.ascontiguousarray(v.reshape(BT, DM).astype(bf16).T)
    woT = np.ascontiguousarray(Wo.astype(bf16).T)
    in_maps = []
    for c in range(N_CORES):
        sl = slice(c * FL, (c + 1) * FL)
        in_maps.append({
            "qT": qT, "kT": kT, "vT": vT,
            "wqT": np.ascontiguousarray(Wq[sl, :].astype(bf16).T),
            "wkT": np.ascontiguousarray(Wk[sl, :].astype(bf16).T),
            "wvT": np.ascontiguousarray(Wv[sl, :].astype(bf16).T),
            "woT": woT,
        })
    return in_maps


def run_spmd(inputs, trace=False):
    if "nc" not in _CACHE:
        _CACHE["nc"] = _build()
    nc = _CACHE["nc"]
    in_maps = _prep_inputs(**inputs)
    res = run_bass_kernel_spmd(nc, in_maps, core_ids=list(range(N_CORES)),
                               trace=trace)
    o = np.empty((BT, DM), dtype=np.float32)
    for c in range(N_CORES):
        o[c * TB:(c + 1) * TB, :] = res.results[c]["outT"].T
    return o.reshape(B, S, DM), res


def kernel(q, k, v, Wq, Wk, Wv, Wo):
    o, _ = run_spmd(dict(q=q, k=k, v=v, Wq=Wq, Wk=Wk, Wv=Wv, Wo=Wo))
    return o


# revision 15
# speedup vs baseline: 1.0443x; 1.0057x over previous
"""Multi-head attention (B=2, S=2048, H=32, D=128) on 8 Trainium2 NeuronCores.

Sharding: tensor-parallel across heads.
  - core c owns heads [4c, 4c+4) (= feature slice [512c, 512(c+1)) of the
    projected dim) for BOTH batches.
  - Each core projects q/k/v (weights column-sharded by head) for all
    4096 tokens, runs attention for its 4 heads (batches never mix), then
    an 8-core AllToAll (split 2/1/1 over heads so transfers overlap the
    attention tail and the output projection start) reshards the context
    from head-major to token-major (512 tokens per core, global token
    index = b*2048 + s). Each core runs the full output projection on its
    token block, producing o^T [4096, 512]; host transposes + concatenates.

Layouts (per core, all device compute in bf16 with fp32 PSUM accumulation):
  - inputs arrive pre-transposed feature-major: qT/kT/vT [4096, 4096]
  - qpT/kpT [128, 4 heads, 4096 tokens] feature-major
  - vp [128, 32 token-tiles, 512 features] token-major
  - scores computed k-major S^T[k, q] in 2-bank PSUM tiles so each Exp
    activation covers 1024 columns (amortizes ACT instruction overhead);
    exp(P^T) feeds the ctx matmul directly as rhs (no transposes anywhere)
  - softmax row sums: DVE adds the 16 P^T tiles, then one ones[128,128]
    matmul fuses the partition-sum with the broadcast of s across
    partitions; 1/s (reciprocal_approx_fast) applied during ctx evacuation.
"""

import numpy as np
import ml_dtypes

import concourse.bacc as bacc
import concourse.mybir as mybir
import concourse.tile as tile
from concourse.bass_utils import run_bass_kernel_spmd

bf16 = ml_dtypes.bfloat16

B, S, H, D = 2, 2048, 32, 128
DM = H * D                      # 4096
BT = B * S                      # 4096 tokens total
N_CORES = 8
HL = H // N_CORES               # heads per core = 4
FL = HL * D                     # feature slice per core = 512
TB = BT // N_CORES              # output token block per core = 512
SCALE = float(D) ** -0.5

F32 = mybir.dt.float32
BF16 = mybir.dt.bfloat16

_CACHE = {}


def _build():
    nc = bacc.Bacc("TRN2", target_bir_lowering=False, debug=False,
                   num_devices=N_CORES)

    qT = nc.dram_tensor("qT", [DM, BT], BF16, kind="ExternalInput")
    kT = nc.dram_tensor("kT", [DM, BT], BF16, kind="ExternalInput")
    vT = nc.dram_tensor("vT", [DM, BT], BF16, kind="ExternalInput")
    wqT = nc.dram_tensor("wqT", [DM, FL], BF16, kind="ExternalInput")
    wkT = nc.dram_tensor("wkT", [DM, FL], BF16, kind="ExternalInput")
    wvT = nc.dram_tensor("wvT", [DM, FL], BF16, kind="ExternalInput")
    woT = nc.dram_tensor("woT", [DM, DM], BF16, kind="ExternalInput")
    outT = nc.dram_tensor("outT", [DM, TB], F32, kind="ExternalOutput")

    NKT = S // 128              # 16 k token-tiles per sequence
    Exp = mybir.ActivationFunctionType.Exp
    # AllToAll split over local heads: {0,1}, {2}, {3}
    A2A_H = [(0, 2), (2, 3), (3, 4)]

    with tile.TileContext(nc) as tc:
        with (
            tc.tile_pool(name="persist", bufs=1) as persist,
            tc.tile_pool(name="big", bufs=3, space="PSUM") as bigp,
            tc.tile_pool(name="ps", bufs=2, space="PSUM") as psp,
            tc.tile_pool(name="dram", bufs=1, space="DRAM") as dram,
        ):
            qpT = persist.tile([128, HL, BT], BF16, tag="qpT")
            kpT = persist.tile([128, HL, BT], BF16, tag="kpT")
            vp = persist.tile([128, BT // 128, FL], BF16, tag="vp")

            ones_m = persist.tile([128, 128], BF16, tag="ones_m")
            nc.vector.memset(ones_m[:], 1.0)

            # ---------------- phase 1: projections ----------------
            with tc.tile_pool(name="proj", bufs=2) as proj:
                for x_dram, w_dram, feat_major, out_t in (
                    (qT, wqT, True, qpT),
                    (kT, wkT, True, kpT),
                    (vT, wvT, False, vp),
                ):
                    x_r = x_dram.ap().rearrange("(kk p) t -> p kk t", p=128)
                    w_r = w_dram.ap().rearrange("(kk p) f -> p kk f", p=128)
                    wh = proj.tile([128, 32, FL], BF16, tag="w")
                    # split the weight load so the first matmuls start sooner
                    nc.sync.dma_start(out=wh[:, 0:16, :],
                                      in_=w_r[:, 0:16, :])
                    nc.sync.dma_start(out=wh[:, 16:32, :],
                                      in_=w_r[:, 16:32, :])
                    for tch in range(8):       # 512-token chunks
                        pss = [bigp.tile([128, 1024], F32, tag="big",
                                         name=f"pss{mp}")
                               for mp in range(2)]
                        for kh in range(2):    # halves of the contraction
                            xs = proj.tile([128, 16, 512], BF16, tag="xs")
                            nc.sync.dma_start(
                                out=xs[:],
                                in_=x_r[:, kh * 16:(kh + 1) * 16,
                                        tch * 512:(tch + 1) * 512])
                            for ms in range(4):
                                dst = pss[ms // 2][:, (ms % 2) * 512:
                                                   (ms % 2 + 1) * 512]
                                for kk in range(16):
                                    first = (kh == 0 and kk == 0)
                                    last = (kh == 1 and kk == 15)
                                    if feat_major:
                                        # out[f_out, t] += W^T.T @ xT
                                        nc.tensor.matmul(
                                            dst,
                                            wh[:, kh * 16 + kk,
                                               ms * 128:(ms + 1) * 128],
                                            xs[:, kk, :],
                                            start=first, stop=last)
                                    else:
                                        # out[t, f_out] += xT.T @ W^T
                                        nc.tensor.matmul(
                                            dst,
                                            xs[:, kk,
                                               ms * 128:(ms + 1) * 128],
                                            wh[:, kh * 16 + kk, :],
                                            start=first, stop=last)
                        for mp in range(2):
                            if feat_major:
                                dst = out_t[:, 2 * mp:2 * mp + 2,
                                            tch * 512:(tch + 1) * 512]
                            else:
                                dst = out_t[:, tch * 4 + 2 * mp:
                                            tch * 4 + 2 * mp + 2, :]
                            nc.vector.tensor_copy(dst, pss[mp][:])

            # ---------------- phase 2: attention ----------------
            in_bufs, out_bufs = [], []
            for gi, (h0, h1) in enumerate(A2A_H):
                in_bufs.append(dram.tile([N_CORES, h1 - h0, 128, TB], BF16,
                                         name=f"a2a_in{gi}"))
                out_bufs.append(dram.tile([N_CORES, h1 - h0, 128, TB], BF16,
                                          name=f"a2a_out{gi}"))

            with tc.tile_pool(name="attn", bufs=2) as attn:
                for hl in range(HL):
                    for b in range(B):
                        for qb in range(4):    # 512-query blocks
                            qs = slice(b * S + qb * TB, b * S + (qb + 1) * TB)
                            pt = attn.tile([128, NKT, TB], BF16, tag="pt",
                                           bufs=3)
                            for g in range(NKT // 2):   # k-tile pairs
                                st = bigp.tile([128, 1024], F32, tag="big")
                                for half in range(2):
                                    kt = 2 * g + half
                                    # S^T[k_tok, q] = khT.T @ qhT
                                    nc.tensor.matmul(
                                        st[:, half * 512:(half + 1) * 512],
                                        kpT[:, hl, b * S + kt * 128:
                                            b * S + (kt + 1) * 128],
                                        qpT[:, hl, qs],
                                        start=True, stop=True)
                                nc.scalar.activation(
                                    pt[:, 2 * g:2 * g + 2, :], st[:],
                                    Exp, scale=SCALE)
                            # partial row sums over the 16 k-tiles (DVE)
                            sp2 = attn.tile([128, 2, TB], BF16, tag="sp2",
                                            bufs=3)
                            nc.vector.tensor_add(sp2[:], pt[:, 0:2, :],
                                                 pt[:, 2:4, :])
                            for g in range(2, NKT // 2):
                                nc.vector.tensor_add(sp2[:], sp2[:],
                                                     pt[:, 2 * g:2 * g + 2, :])
                            sp = attn.tile([128, TB], BF16, tag="sp",
                                           bufs=3)
                            nc.vector.tensor_add(sp[:], sp2[:, 0, :],
                                                 sp2[:, 1, :])
                            # fused partition-sum + broadcast: ones.T @ sp
                            ps_b = psp.tile([128, TB], F32, tag="mm")
                            nc.tensor.matmul(ps_b[:], ones_m[:], sp[:],
                                             start=True, stop=True)
                            rsb = attn.tile([128, TB], F32, tag="rsb",
                                            bufs=3)
                            nc.vector.reciprocal_approx_fast(rsb[:], ps_b[:])
                            # ctx^T[d, q] = sum_kt vh[kt].T @ P^T[kt]
                            ps_c = psp.tile([128, TB], F32, tag="mm")
                            for kt in range(NKT):
                                nc.tensor.matmul(
                                    ps_c[:],
                                    vp[:, b * NKT + kt,
                                       hl * 128:(hl + 1) * 128],
                                    pt[:, kt, :],
                                    start=(kt == 0), stop=(kt == NKT - 1))
                            ctxs = attn.tile([128, TB], BF16, tag="ctxs",
                                             bufs=3)
                            nc.vector.tensor_tensor(
                                ctxs[:], ps_c[:], rsb[:],
                                op=mybir.AluOpType.mult)
                            for gi, (h0, h1) in enumerate(A2A_H):
                                if h0 <= hl < h1:
                                    nc.sync.dma_start(
                                        out=in_bufs[gi][b * 4 + qb, hl - h0],
                                        in_=ctxs[:])
                    for gi, (h0, h1) in enumerate(A2A_H):
                        if hl == h1 - 1:
                            nc.gpsimd.collective_compute(
                                "AllToAll", mybir.AluOpType.bypass,
                                replica_groups=[list(range(N_CORES))],
                                ins=[in_bufs[gi].opt()],
                                outs=[out_bufs[gi].opt()])

            # ---------------- phase 3: output projection ----------------
            # o^T[f_out, t] = sum over the 32 ctx feature tiles
            # global feature tile kk = j*HL + hl  (j = source rank)
            with tc.tile_pool(name="oproj", bufs=2) as op:
                ctxg = []
                for gi, (h0, h1) in enumerate(A2A_H):
                    cg = op.tile([128, N_CORES * (h1 - h0), TB], BF16,
                                 tag=f"ctxg{gi}", name=f"ctxg{gi}")
                    nc.sync.dma_start(
                        out=cg[:],
                        in_=out_bufs[gi].rearrange("j h p t -> p (j h) t"))
                    ctxg.append(cg)
                # accumulation order: all group-0 tiles, then 1, then 2 so
                # early matmuls run while later AllToAlls are in flight
                mm_seq = []
                for gi, (h0, h1) in enumerate(A2A_H):
                    for j in range(N_CORES):
                        for hl in range(h0, h1):
                            kk = j * HL + hl
                            mm_seq.append(
                                (kk, ctxg[gi], j * (h1 - h0) + hl - h0))

                wo_r = woT.ap().rearrange("(kk p) f -> p kk f", p=128)
                for fop in range(16):          # 256-wide out-feature pairs
                    woc = op.tile([128, 32, 256], BF16, tag="woc")
                    nc.sync.dma_start(
                        out=woc[:], in_=wo_r[:, :, fop * 256:(fop + 1) * 256])
                    ps_o = bigp.tile([128, 1024], F32, tag="big")
                    for sub in range(2):
                        dst = ps_o[:, sub * 512:(sub + 1) * 512]
                        for n_mm, (kk, cg, ci) in enumerate(mm_seq):
                            nc.tensor.matmul(
                                dst,
                                woc[:, kk, sub * 128:(sub + 1) * 128],
                                cg[:, ci, :],
                                start=(n_mm == 0), stop=(n_mm == 31))
                    ot = op.tile([128, 2, TB], F32, tag="ot")
                    nc.vector.tensor_copy(ot[:], ps_o[:])
                    nc.sync.dma_start(
                        out=outT.ap().rearrange(
                            "(fo p) t -> p fo t", p=128)[
                            :, fop * 2:fop * 2 + 2, :],
                        in_=ot[:])

    nc.compile()
    return nc


def _prep_inputs(q, k, v, Wq, Wk, Wv, Wo):
    """Host-side sharding: cast to bf16, transpose to feature-major, slice."""
    q, k, v = (np.asarray(x, dtype=np.float32) for x in (q, k, v))
    Wq, Wk, Wv, Wo = (np.asarray(x, dtype=np.float32)
                      for x in (Wq, Wk, Wv, Wo))
    qT = np.ascontiguousarray(q.reshape(BT, DM).astype(bf16).T)
    kT = np.ascontiguousarray(k.reshape(BT, DM).astype(bf16).T)
    vT = np.ascontiguousarray(v.reshape(BT, DM).astype(bf16).T)
    woT = np.ascontiguousarray(Wo.astype(bf16).T)
    in_maps = []
    for c in range(N_CORES):
        sl = slice(c * FL, (c + 1) * FL)
        in_maps.append({
            "qT": qT, "kT": kT, "vT": vT,
            "wqT": np.ascontiguousarray(Wq[sl, :].astype(bf16).T),
            "wkT": np.ascontiguousarray(Wk[sl, :].astype(bf16).T),
            "wvT": np.ascontiguousarray(Wv[sl, :].astype(bf16).T),
            "woT": woT,
        })
    return in_maps


def run_spmd(inputs, trace=False):
    if "nc" not in _CACHE:
        _CACHE["nc"] = _build()
    nc = _CACHE["nc"]
    in_maps = _prep_inputs(**inputs)
    res = run_bass_kernel_spmd(nc, in_maps, core_ids=list(range(N_CORES)),
                               trace=trace)
    o = np.empty((BT, DM), dtype=np.float32)
    for c in range(N_CORES):
        o[c * TB:(c + 1) * TB, :] = res.results[c]["outT"].T
    return o.reshape(B, S, DM), res


def kernel(q, k, v, Wq, Wk, Wv, Wo):
    o, _ = run_spmd(dict(q=q, k=k, v=v, Wq=Wq, Wk=Wk, Wv=Wv, Wo=Wo))
    return o
